# revision 8
# baseline (speedup 1.0000x reference)
"""Trainium2 Bass kernel for the CPC contrastive loss problem.

Math (reference):
    fx = relu(x @ W1 + b1) @ W2 + b2          [N, Z]
    fz = z @ Wz + bz                          [N, Z]
    u[n] = fx[n] @ Ws[c[n]]                   [N, Z]
    T = softplus(<u, fz>_row)                 [N]
    neg_T[i] = mean_{j: c[j]==c[i]} softplus(<u[i], fz[j]>)
    out = log(T + eps) - log(neg_T + eps)

Key optimization: neg_T[i] only involves same-category j's (~N/C = 128 of
8192), so rows are grouped by category on the host and S is computed in
per-category blocks instead of the full NxN matrix (64x less work).

Sharding: 64 categories -> 8 cores x 8 categories. Each category's rows are
zero-padded to a B=256 bucket (category sizes are Binomial(8192, 1/64);
max ~165 in practice; a numpy fallback covers overflow). The j-width of
each S block is trimmed to JW=192 since only n_k <= JW columns are real.
Padded fz columns are exactly zero (bias applied via a masked rank-1
matmul), so each padded column contributes exactly softplus(0)=ln2,
subtracted out with a host-precomputed correction.

Precision strategy:
  - MLP / u chain in float32r matmuls (tf32-like, ~1.5e-4): feeds both S
    and the diagonal dots d, whose error propagates ~1:1 into the output.
  - S-blocks in bf16 (~2.3e-3 on S): only enters through a 128-term
    softplus AVERAGE inside a log, contributing ~3e-4.
  - softplus(S) = relu(S) + log1p(exp(-|S|)) split: never under/overflows
    (sigmoid/softplus LUTs are unavailable or underflow at |S|~500).
  - The T term log(softplus(d) + 1e-8) needs ~1e-9 ABSOLUTE accuracy when
    d is very negative: computed on [128,16] via exp + branch-free log1p
    (series for e < 1e-4, Ln(1+e) otherwise).
"""

import sys

for _p in ("/opt/trn_rl_repo", "/root/.axon_site/_ro/trn_rl_repo"):
    if _p not in sys.path:
        sys.path.append(_p)

import numpy as np

import concourse.bacc as bacc
import concourse.tile as tile
from concourse import mybir as mb
from concourse.bass_utils import run_bass_kernel_spmd

# ---------------------------------------------------------------- constants
N, IN, Z, C, H = 8192, 512, 128, 64, 50
NCORES = 8
G = C // NCORES          # categories per core
B = 256                  # bucket (padded category) size, i-direction
JW = 192                 # j-width of S blocks (max category size <= JW)
R = G * B                # padded rows per core = 2048
NCHUNK = R // 128        # 16 row-chunks of 128
KX = IN // 128           # 4 k-tiles for x
NT = R // 512            # 4 n-tiles of 512
NH = 4                   # S-stage groups (PSUM-sized)
CPH = G // NH            # categories per group
EPS = 1e-8
LN2 = float(np.log(2.0))

F = mb.ActivationFunctionType
OP = mb.AluOpType
FP32 = mb.dt.float32
FP32R = mb.dt.float32r
BF16 = mb.dt.bfloat16

_PROGRAM = None


def _build_program():
    nc = bacc.Bacc("TRN2", target_bir_lowering=False, debug=False)

    d_xgT = nc.dram_tensor("xgT", [IN, R], FP32, kind="ExternalInput").ap()
    d_zgT = nc.dram_tensor("zgT", [Z, R], FP32, kind="ExternalInput").ap()
    d_mrow = nc.dram_tensor("mrow", [1, R], FP32, kind="ExternalInput").ap()
    d_wsg = nc.dram_tensor("wsg", [G * Z, Z], FP32, kind="ExternalInput").ap()
    d_w1 = nc.dram_tensor("w1", [IN, H], FP32, kind="ExternalInput").ap()
    d_b1 = nc.dram_tensor("b1", [H, 1], FP32, kind="ExternalInput").ap()
    d_w2 = nc.dram_tensor("w2", [H, Z], FP32, kind="ExternalInput").ap()
    d_b2 = nc.dram_tensor("b2", [Z, 1], FP32, kind="ExternalInput").ap()
    d_wz = nc.dram_tensor("wz", [Z, Z], FP32, kind="ExternalInput").ap()
    d_bz = nc.dram_tensor("bz", [1, Z], FP32, kind="ExternalInput").ap()
    d_subv = nc.dram_tensor("subv", [128, NCHUNK], FP32, kind="ExternalInput").ap()
    d_pinv = nc.dram_tensor("pinv", [128, NCHUNK], FP32, kind="ExternalInput").ap()
    d_yout = nc.dram_tensor("yout", [R], FP32, kind="ExternalOutput").ap()

    with tile.TileContext(nc) as tc:
        with (
            tc.tile_pool(name="const", bufs=1) as const,
            tc.tile_pool(name="junk", bufs=2) as junkp,
            tc.tile_pool(name="psum_mlp", bufs=2, space="PSUM") as psum_mlp,
            tc.tile_pool(name="psum_s", bufs=3, space="PSUM") as psum_sp,
        ):
            # ---- resident inputs (matmul operands typed float32r)
            s_xgT = const.tile([128, KX, R], FP32R)
            x_view = d_xgT.bitcast(FP32R).rearrange("(k p) n -> p k n", p=128)
            for k in range(KX):
                for n in range(NT):
                    ns = slice(n * 512, (n + 1) * 512)
                    nc.sync.dma_start(out=s_xgT[:, k, ns], in_=x_view[:, k, ns])
            s_zgT = const.tile([128, R], FP32R)
            nc.sync.dma_start(out=s_zgT[:], in_=d_zgT.bitcast(FP32R)[:])
            s_mrow = const.tile([1, R], FP32R)
            nc.sync.dma_start(out=s_mrow[:], in_=d_mrow.bitcast(FP32R)[:])
            s_wsg = const.tile([128, G, Z], FP32R)
            nc.sync.dma_start(
                out=s_wsg[:],
                in_=d_wsg.bitcast(FP32R).rearrange("(g p) c -> p g c", p=128),
            )
            s_w1 = const.tile([128, KX, H], FP32R)
            nc.sync.dma_start(
                out=s_w1[:], in_=d_w1.bitcast(FP32R).rearrange("(k p) h -> p k h", p=128)
            )
            s_b1 = const.tile([H, 1], FP32)
            nc.sync.dma_start(out=s_b1[:], in_=d_b1[:])
            s_w2 = const.tile([H, Z], FP32R)
            nc.sync.dma_start(out=s_w2[:], in_=d_w2.bitcast(FP32R)[:])
            s_b2 = const.tile([Z, 1], FP32)
            nc.sync.dma_start(out=s_b2[:], in_=d_b2[:])
            s_wz = const.tile([128, Z], FP32R)
            nc.sync.dma_start(out=s_wz[:], in_=d_wz.bitcast(FP32R)[:])
            s_bz = const.tile([1, Z], FP32R)
            nc.sync.dma_start(out=s_bz[:], in_=d_bz.bitcast(FP32R)[:])
            s_subv = const.tile([128, NCHUNK], FP32)
            nc.sync.dma_start(out=s_subv[:], in_=d_subv[:])
            s_pinv = const.tile([128, NCHUNK], FP32)
            nc.sync.dma_start(out=s_pinv[:], in_=d_pinv[:])
            s_ones = const.tile([128, 1], FP32)
            nc.vector.memset(s_ones[:], 1.0)
            s_eps = const.tile([128, 1], FP32)
            nc.vector.memset(s_eps[:], EPS)

            # ---- persistent intermediates
            s_h1T = const.tile([H, R], FP32R)
            s_fxT = const.tile([128, R], FP32R)
            s_fzT = const.tile([128, R], FP32R)
            s_uT = const.tile([128, R], FP32R)
            s_u16 = const.tile([128, R], BF16)
            s_fz16 = const.tile([128, R], BF16)
            s_prod = const.tile([128, R], FP32)
            s_pos = const.tile([128, NCHUNK], FP32)

            # ---- stage 1: h1T = relu(W1^T @ xgT + b1)   [50, R]
            for n in range(NT):
                ns = slice(n * 512, (n + 1) * 512)
                ph = psum_mlp.tile([H, 512], FP32, tag="mlp")
                for k in range(KX):
                    nc.tensor.matmul(
                        ph[:],
                        lhsT=s_w1[:, k, :],
                        rhs=s_xgT[:, k, ns],
                        start=(k == 0),
                        stop=(k == KX - 1),
                    )
                nc.scalar.activation(
                    out=s_h1T[:, ns], in_=ph[:], func=F.Relu, bias=s_b1[:], scale=1.0
                )

            # ---- stage 2: fxT = W2^T @ h1T + b2   [128, R]
            for n in range(NT):
                ns = slice(n * 512, (n + 1) * 512)
                pf = psum_mlp.tile([128, 512], FP32, tag="mlp")
                nc.tensor.matmul(
                    pf[:], lhsT=s_w2[:], rhs=s_h1T[:, ns], start=True, stop=True
                )
                nc.any.tensor_scalar_add(s_fxT[:, ns], pf[:], s_b2[:])

            # ---- stage 3: fzT = Wz^T @ zgT + bz x mrow   [128, R]
            # (rank-1 bias keeps padded columns exactly zero)
            for n in range(NT):
                ns = slice(n * 512, (n + 1) * 512)
                pz = psum_mlp.tile([128, 512], FP32, tag="mlp")
                nc.tensor.matmul(
                    pz[:], lhsT=s_wz[:], rhs=s_zgT[:, ns], start=True, stop=False
                )
                nc.tensor.matmul(
                    pz[:], lhsT=s_bz[:], rhs=s_mrow[:, ns], start=False, stop=True
                )
                nc.any.tensor_copy(s_fzT[:, ns], pz[:])
                nc.gpsimd.tensor_copy(s_fz16[:, ns], s_fzT.bitcast(FP32)[:, ns])

            # ---- stage 4: uT[:, g-block] = Ws[g]^T @ fxT[:, g-block]
            for g in range(G):
                gs = slice(g * B, (g + 1) * B)
                pu = psum_mlp.tile([128, B], FP32, tag="mlp")
                nc.tensor.matmul(
                    pu[:], lhsT=s_wsg[:, g, :], rhs=s_fxT[:, gs], start=True, stop=True
                )
                nc.any.tensor_copy(s_uT[:, gs], pu[:])
                nc.gpsimd.tensor_copy(s_u16[:, gs], s_uT.bitcast(FP32)[:, gs])

            # ---- stage 5: d = per-row <u, fz> in [128, NCHUNK] chunk layout
            nc.vector.tensor_mul(
                s_prod[:], s_uT.bitcast(FP32)[:], s_fzT.bitcast(FP32)[:]
            )
            pd = psum_mlp.tile([128, NCHUNK], FP32, tag="mlp")
            for ci in range(NCHUNK):
                nc.tensor.matmul(
                    pd[:, ci : ci + 1],
                    lhsT=s_prod[:, ci * 128 : (ci + 1) * 128],
                    rhs=s_ones[:],
                    start=True,
                    stop=True,
                )
            s_d = const.tile([128, NCHUNK], FP32)
            nc.vector.tensor_copy(s_d[:], pd[:])

            # ---- stage 6: S blocks (bf16) + softplus row-sums
            # softplus(S) = relu(S) + log1p(exp(-|S|));
            # row-sum accumulated per chunk into s_pos.
            for grp in range(NH):
                # chunk stride padded to 256 so no matmul crosses a PSUM bank
                pS = psum_sp.tile([128, CPH * 2, 256], FP32, tag="spsum")
                for gg in range(CPH):
                    g = grp * CPH + gg
                    for h in range(2):
                        nc.tensor.matmul(
                            pS[:, 2 * gg + h, :JW],
                            lhsT=s_u16[:, g * B + h * 128 : g * B + (h + 1) * 128],
                            rhs=s_fz16[:, g * B : g * B + JW],
                            start=True,
                            stop=True,
                        )
                nch = CPH * 2
                s_a = junkp.tile([128, nch * JW], FP32, tag="abs")
                nc.vector.tensor_scalar(
                    out=s_a.rearrange("p (c j) -> p c j", j=JW),
                    in0=pS[:, :, :JW],
                    scalar1=0.0, scalar2=None, op0=OP.abs_max,
                )
                s_e = junkp.tile([128, nch * JW], FP32, tag="exp")
                nc.scalar.activation(out=s_e[:], in_=s_a[:], func=F.Exp, scale=-1.0)
                s_l = junkp.tile([128, nch * JW], FP32, tag="l1p")
                nc.scalar.activation(out=s_l[:], in_=s_e[:], func=F.Ln, bias=1.0)
                for ch in range(nch):
                    ci = grp * nch + ch
                    jk = junkp.tile([128, JW], FP32, tag="junk")
                    nc.vector.scalar_tensor_tensor(
                        out=jk[:],
                        in0=pS[:, ch, :JW],
                        scalar=0.0,
                        in1=s_l[:, ch * JW : (ch + 1) * JW],
                        op0=OP.max,
                        op1=OP.add,
                        accum_out=s_pos[:, ci : ci + 1],
                    )

            # ---- stage 7: accurate T-term: softplus(d) via exp/log1p
            sm = const
            s_ad = sm.tile([128, NCHUNK], FP32)
            nc.scalar.activation(out=s_ad[:], in_=s_d[:], func=F.Abs)
            s_ed = sm.tile([128, NCHUNK], FP32)
            nc.scalar.activation(out=s_ed[:], in_=s_ad[:], func=F.Exp, scale=-1.0)
            s_big = sm.tile([128, NCHUNK], FP32)
            nc.scalar.activation(out=s_big[:], in_=s_ed[:], func=F.Ln, bias=1.0)
            s_t1 = sm.tile([128, NCHUNK], FP32)
            nc.vector.tensor_scalar(
                out=s_t1[:], in0=s_ed[:], scalar1=-0.5, scalar2=1.0,
                op0=OP.mult, op1=OP.add,
            )  # 1 - e/2
            s_ssm = sm.tile([128, NCHUNK], FP32)
            nc.vector.tensor_mul(s_ssm[:], s_ed[:], s_t1[:])  # e*(1 - e/2)
            s_mk = sm.tile([128, NCHUNK], FP32)
            nc.vector.tensor_scalar(
                out=s_mk[:], in0=s_ed[:], scalar1=1e-4, scalar2=None, op0=OP.is_lt
            )
            s_l1p = sm.tile([128, NCHUNK], FP32)
            nc.vector.select(s_l1p[:], s_mk[:], s_ssm[:], s_big[:])
            s_T0 = sm.tile([128, NCHUNK], FP32)
            nc.vector.scalar_tensor_tensor(
                out=s_T0[:], in0=s_d[:], scalar=0.0, in1=s_l1p[:],
                op0=OP.max, op1=OP.add,
            )  # relu(d) + log1p(exp(-|d|))
            s_logT = sm.tile([128, NCHUNK], FP32)
            nc.scalar.activation(out=s_logT[:], in_=s_T0[:], func=F.Ln, bias=s_eps[:])

            # ---- stage 8: neg_T and final output
            s_q1 = sm.tile([128, NCHUNK], FP32)
            nc.vector.tensor_sub(s_q1[:], s_pos[:], s_subv[:])
            s_q2 = sm.tile([128, NCHUNK], FP32)
            nc.vector.tensor_mul(s_q2[:], s_q1[:], s_pinv[:])
            s_q2c = sm.tile([128, NCHUNK], FP32)
            nc.vector.tensor_scalar_max(s_q2c[:], s_q2[:], 0.0)
            s_logn = sm.tile([128, NCHUNK], FP32)
            nc.scalar.activation(
                out=s_logn[:], in_=s_q2c[:], func=F.Ln, bias=s_eps[:]
            )
            s_y = sm.tile([128, NCHUNK], FP32)
            nc.vector.tensor_sub(s_y[:], s_logT[:], s_logn[:])

            nc.sync.dma_start(
                out=d_yout.rearrange("(c p) -> p c", p=128), in_=s_y[:]
            )

    nc.compile()
    return nc


def get_program():
    global _PROGRAM
    if _PROGRAM is None:
        _PROGRAM = _build_program()
    return _PROGRAM


# ---------------------------------------------------------------- host side
def _prep_core_inputs(x, z, Ws, idx_lists, core):
    """Build the per-core input map (grouped, padded, transposed)."""
    xgT = np.zeros((IN, R), np.float32)
    zgT = np.zeros((Z, R), np.float32)
    mrow = np.zeros((1, R), np.float32)
    subv = np.zeros(R, np.float32)
    pinv = np.zeros(R, np.float32)
    for s in range(G):
        k = core * G + s
        idx = idx_lists[k]
        n = len(idx)
        lo = s * B
        if n:
            xgT[:, lo : lo + n] = x[idx].T
            zgT[:, lo : lo + n] = z[idx].T
            mrow[0, lo : lo + n] = 1.0
            subv[lo : lo + B] = (JW - n) * LN2
            pinv[lo : lo + B] = 1.0 / n
    wsg = np.ascontiguousarray(
        Ws[core * G : (core + 1) * G].reshape(G * Z, Z), dtype=np.float32
    )
    # chunk layout [128, NCHUNK]: row r = ci*128 + p  ->  [p, ci]
    subv = np.ascontiguousarray(subv.reshape(NCHUNK, 128).T)
    pinv = np.ascontiguousarray(pinv.reshape(NCHUNK, 128).T)
    return {
        "xgT": xgT, "zgT": zgT, "mrow": mrow, "wsg": wsg,
        "subv": subv, "pinv": pinv,
    }


def _numpy_fallback(x, c, z, W1, b1, W2, b2, Wz, bz, Ws):
    x64 = x.astype(np.float64)
    fx = np.maximum(x64 @ W1.astype(np.float64) + b1, 0.0) @ W2.astype(
        np.float64
    ) + b2
    fz = z.astype(np.float64) @ Wz.astype(np.float64) + bz
    u = np.einsum("nd,nde->ne", fx, Ws.astype(np.float64)[c])

    def sp(v):
        return np.log1p(np.exp(-np.abs(v))) + np.maximum(v, 0.0)

    T = sp(np.einsum("ne,ne->n", u, fz))
    out = np.empty(N, np.float64)
    for k in range(C):
        idx = np.where(c == k)[0]
        if len(idx) == 0:
            continue
        Sk = sp(u[idx] @ fz[idx].T)
        neg = Sk.mean(axis=1)
        out[idx] = np.log(T[idx] + EPS) - np.log(neg + EPS)
    return out.astype(np.float32)


def kernel(x, c, z, W1, b1, W2, b2, Wz, bz, Ws):
    x = np.ascontiguousarray(np.asarray(x), dtype=np.float32)
    z = np.ascontiguousarray(np.asarray(z), dtype=np.float32)
    W1 = np.ascontiguousarray(np.asarray(W1), dtype=np.float32)
    b1 = np.ascontiguousarray(np.asarray(b1), dtype=np.float32)
    W2 = np.ascontiguousarray(np.asarray(W2), dtype=np.float32)
    b2 = np.ascontiguousarray(np.asarray(b2), dtype=np.float32)
    Wz = np.ascontiguousarray(np.asarray(Wz), dtype=np.float32)
    bz = np.ascontiguousarray(np.asarray(bz), dtype=np.float32)
    Ws = np.ascontiguousarray(np.asarray(Ws), dtype=np.float32)
    cf = np.asarray(c).reshape(-1).astype(np.int64)

    idx_lists = [np.where(cf == k)[0] for k in range(C)]
    if max(len(i) for i in idx_lists) > JW:
        return _numpy_fallback(x, cf, z, W1, b1, W2, b2, Wz, bz, Ws)

    shared = {
        "w1": W1,
        "b1": b1.reshape(H, 1),
        "w2": W2,
        "b2": b2.reshape(Z, 1),
        "wz": Wz,
        "bz": bz.reshape(1, Z),
    }
    in_maps = []
    for core in range(NCORES):
        m = _prep_core_inputs(x, z, Ws, idx_lists, core)
        m.update(shared)
        in_maps.append(m)

    nc = get_program()
    res = run_bass_kernel_spmd(nc, in_maps, core_ids=list(range(NCORES)))

    out = np.empty(N, np.float32)
    for core in range(NCORES):
        y = res.results[core]["yout"]
        for s in range(G):
            k = core * G + s
            idx = idx_lists[k]
            if len(idx):
                out[idx] = y[s * B : s * B + len(idx)]
    return out


# revision 17
# speedup vs baseline: 1.1156x; 1.1156x over previous
"""Trainium2 Bass kernel for the CPC contrastive loss problem.

Math (reference):
    fx = relu(x @ W1 + b1) @ W2 + b2          [N, Z]
    fz = z @ Wz + bz                          [N, Z]
    u[n] = fx[n] @ Ws[c[n]]                   [N, Z]
    T = softplus(<u, fz>_row)                 [N]
    neg_T[i] = mean_{j: c[j]==c[i]} softplus(<u[i], fz[j]>)
    out = log(T + eps) - log(neg_T + eps)

Key optimization: neg_T[i] only involves same-category j's (~N/C = 128 of
8192), so rows are grouped by category on the host and S is computed in
per-category blocks instead of the full NxN matrix (64x less work).

Sharding: 64 categories -> 8 cores x 8 categories. Each category's rows are
zero-padded to a B=256 bucket (category sizes are Binomial(8192, 1/64);
max ~165 in practice; a numpy fallback covers overflow). The j-width of
each S block is trimmed to JW=192 since only n_k <= JW columns are real.
Padded fz columns are exactly zero (bias applied via a masked rank-1
matmul), so each padded column contributes exactly softplus(0)=ln2,
subtracted out with a host-precomputed correction.

Precision strategy:
  - MLP / u chain in float32r matmuls (tf32-like, ~1.5e-4): feeds both S
    and the diagonal dots d, whose error propagates ~1:1 into the output.
  - S-blocks in bf16 (~2.3e-3 on S): only enters through a 128-term
    softplus AVERAGE inside a log, contributing ~3e-4.
  - softplus(S) = relu(S) + log1p(exp(-|S|)) split: never under/overflows
    (sigmoid/softplus LUTs are unavailable or underflow at |S|~500).
  - The T term log(softplus(d) + 1e-8) needs ~1e-9 ABSOLUTE accuracy when
    d is very negative: computed on [128,16] via exp + branch-free log1p
    (series for e < 1e-4, Ln(1+e) otherwise).
"""

import sys

for _p in ("/opt/trn_rl_repo", "/root/.axon_site/_ro/trn_rl_repo"):
    if _p not in sys.path:
        sys.path.append(_p)

import numpy as np

import concourse.bacc as bacc
import concourse.tile as tile
from concourse import mybir as mb
from concourse.bass_utils import run_bass_kernel_spmd

# ---------------------------------------------------------------- constants
N, IN, Z, C, H = 8192, 512, 128, 64, 50
NCORES = 8
G = C // NCORES          # categories per core
B = 256                  # bucket (padded category) size, i-direction
JW = 192                 # j-width of S blocks (max category size <= JW)
R = G * B                # padded rows per core = 2048
NCHUNK = R // 128        # 16 row-chunks of 128
KX = IN // 128           # 4 k-tiles for x
NT = R // 512            # 4 n-tiles of 512
NH = 4                   # S-stage groups (PSUM-sized)
CPH = G // NH            # categories per group
EPS = 1e-8
LN2 = float(np.log(2.0))

F = mb.ActivationFunctionType
OP = mb.AluOpType
FP32 = mb.dt.float32
FP32R = mb.dt.float32r
BF16 = mb.dt.bfloat16

_PROGRAM = None


def _build_program():
    nc = bacc.Bacc("TRN2", target_bir_lowering=False, debug=False)

    d_xgT = nc.dram_tensor("xgT", [IN, R], FP32, kind="ExternalInput").ap()
    d_zgT = nc.dram_tensor("zgT", [Z, R], FP32, kind="ExternalInput").ap()
    d_mrow = nc.dram_tensor("mrow", [1, R], FP32, kind="ExternalInput").ap()
    d_wsg = nc.dram_tensor("wsg", [G * Z, Z], FP32, kind="ExternalInput").ap()
    d_w1 = nc.dram_tensor("w1", [IN, H], FP32, kind="ExternalInput").ap()
    d_b1 = nc.dram_tensor("b1", [H, 1], FP32, kind="ExternalInput").ap()
    d_w2 = nc.dram_tensor("w2", [H, Z], FP32, kind="ExternalInput").ap()
    d_b2 = nc.dram_tensor("b2", [Z, 1], FP32, kind="ExternalInput").ap()
    d_wz = nc.dram_tensor("wz", [Z, Z], FP32, kind="ExternalInput").ap()
    d_bz = nc.dram_tensor("bz", [1, Z], FP32, kind="ExternalInput").ap()
    d_subv = nc.dram_tensor("subv", [128, NCHUNK], FP32, kind="ExternalInput").ap()
    d_pinv = nc.dram_tensor("pinv", [128, NCHUNK], FP32, kind="ExternalInput").ap()
    d_yout = nc.dram_tensor("yout", [R], FP32, kind="ExternalOutput").ap()

    with tile.TileContext(nc) as tc:
        with (
            tc.tile_pool(name="const", bufs=1) as const,
            tc.tile_pool(name="junk", bufs=2) as junkp,
            tc.tile_pool(name="psum_mlp", bufs=2, space="PSUM") as psum_mlp,
            tc.tile_pool(name="psum_s", bufs=3, space="PSUM") as psum_sp,
        ):
            # ---- small constants first so compute can start ASAP
            s_ones = const.tile([128, 1], FP32)
            nc.vector.memset(s_ones[:], 1.0)
            s_eps = const.tile([128, 1], FP32)
            nc.vector.memset(s_eps[:], EPS)
            # force the natural_log_exp_and_others ACT table load at t=0
            # (every ACT func used below lives in that one set)
            s_warmln = const.tile([128, 1], FP32)
            nc.scalar.activation(out=s_warmln[:], in_=s_ones[:], func=F.Ln, bias=1.0)

            s_w1 = const.tile([128, KX, H], FP32R)
            nc.sync.dma_start(
                out=s_w1[:], in_=d_w1.bitcast(FP32R).rearrange("(k p) h -> p k h", p=128)
            )
            s_b1 = const.tile([H, 1], FP32)
            nc.sync.dma_start(out=s_b1[:], in_=d_b1[:])
            s_w2 = const.tile([H, Z], FP32R)
            nc.sync.dma_start(out=s_w2[:], in_=d_w2.bitcast(FP32R)[:])
            s_b2 = const.tile([Z, 1], FP32)
            nc.sync.dma_start(out=s_b2[:], in_=d_b2[:])
            s_wz = const.tile([128, Z], FP32R)
            nc.sync.dma_start(out=s_wz[:], in_=d_wz.bitcast(FP32R)[:])
            s_bz = const.tile([1, Z], FP32R)
            nc.sync.dma_start(out=s_bz[:], in_=d_bz.bitcast(FP32R)[:])
            s_subv = const.tile([128, NCHUNK], FP32)
            nc.sync.dma_start(out=s_subv[:], in_=d_subv[:])
            s_pinv = const.tile([128, NCHUNK], FP32)
            nc.sync.dma_start(out=s_pinv[:], in_=d_pinv[:])
            s_wsg = const.tile([128, G, Z], FP32R)
            nc.sync.dma_start(
                out=s_wsg[:],
                in_=d_wsg.bitcast(FP32R).rearrange("(g p) c -> p g c", p=128),
            )

            # PE warm-up: cheap junk matmuls so HAM un-throttles before the
            # real work arrives (runs during the input DMA).
            pwarm = psum_mlp.tile([1, 64], FP32, tag="mlp")
            s_wrhs = const.tile([128, 64], FP32)
            nc.vector.memset(s_wrhs[:], 0.0)
            for _ in range(40):
                nc.tensor.matmul(
                    pwarm[:], lhsT=s_ones[:], rhs=s_wrhs[:], start=True, stop=True
                )

            # ---- big inputs, in first-use order (matmul operands float32r)
            s_xgT = const.tile([128, KX, R], FP32R)
            x_view = d_xgT.bitcast(FP32R).rearrange("(k p) n -> p k n", p=128)
            for n in range(NT):
                ns = slice(n * 512, (n + 1) * 512)
                for k in range(KX):
                    nc.sync.dma_start(out=s_xgT[:, k, ns], in_=x_view[:, k, ns])
            s_zgT = const.tile([128, R], FP32R)
            nc.sync.dma_start(out=s_zgT[:], in_=d_zgT.bitcast(FP32R)[:])
            s_mrow = const.tile([1, R], FP32R)
            nc.sync.dma_start(out=s_mrow[:], in_=d_mrow.bitcast(FP32R)[:])

            # ---- persistent intermediates
            s_h1T = const.tile([H, R], FP32R)
            s_fxT = const.tile([128, R], FP32R)
            s_fzT = const.tile([128, R], FP32R)
            s_uT = const.tile([128, R], FP32R)
            s_u16 = const.tile([128, R], BF16)
            s_fz16 = const.tile([128, R], BF16)
            s_prod = const.tile([128, R], FP32)
            s_pos = const.tile([128, NCHUNK], FP32)

            # ---- stage 1: h1T = relu(W1^T @ xgT + b1)   [50, R]
            for n in range(NT):
                ns = slice(n * 512, (n + 1) * 512)
                ph = psum_mlp.tile([H, 512], FP32, tag="mlp")
                for k in range(KX):
                    nc.tensor.matmul(
                        ph[:],
                        lhsT=s_w1[:, k, :],
                        rhs=s_xgT[:, k, ns],
                        start=(k == 0),
                        stop=(k == KX - 1),
                    )
                nc.scalar.activation(
                    out=s_h1T[:, ns], in_=ph[:], func=F.Relu, bias=s_b1[:], scale=1.0
                )

            # ---- stage 2: fxT = W2^T @ h1T + b2   [128, R]
            for n in range(NT):
                ns = slice(n * 512, (n + 1) * 512)
                pf = psum_mlp.tile([128, 512], FP32, tag="mlp")
                nc.tensor.matmul(
                    pf[:], lhsT=s_w2[:], rhs=s_h1T[:, ns], start=True, stop=True
                )
                nc.vector.tensor_scalar_add(s_fxT[:, ns], pf[:], s_b2[:])

            # ---- stage 3: fzT = Wz^T @ zgT + bz x mrow   [128, R]
            # (rank-1 bias keeps padded columns exactly zero)
            for n in range(NT):
                ns = slice(n * 512, (n + 1) * 512)
                pz = psum_mlp.tile([128, 512], FP32, tag="mlp")
                nc.tensor.matmul(
                    pz[:], lhsT=s_wz[:], rhs=s_zgT[:, ns], start=True, stop=False
                )
                nc.tensor.matmul(
                    pz[:], lhsT=s_bz[:], rhs=s_mrow[:, ns], start=False, stop=True
                )
                nc.scalar.copy(s_fzT[:, ns], pz[:])
                nc.gpsimd.tensor_copy(s_fz16[:, ns], s_fzT.bitcast(FP32)[:, ns])

            # ---- stage 4: uT[:, g-block] = Ws[g]^T @ fxT[:, g-block]
            for g in range(G):
                gs = slice(g * B, (g + 1) * B)
                pu = psum_mlp.tile([128, B], FP32, tag="mlp")
                nc.tensor.matmul(
                    pu[:], lhsT=s_wsg[:, g, :], rhs=s_fxT[:, gs], start=True, stop=True
                )
                nc.vector.tensor_copy(s_uT[:, gs], pu[:])
                nc.gpsimd.tensor_copy(s_u16[:, gs], s_uT.bitcast(FP32)[:, gs])

            # ---- stage 5: d = per-row <u, fz> in [128, NCHUNK] chunk layout
            nc.vector.tensor_mul(
                s_prod[:], s_uT.bitcast(FP32)[:], s_fzT.bitcast(FP32)[:]
            )
            pd = psum_mlp.tile([128, NCHUNK], FP32, tag="mlp")
            for ci in range(NCHUNK):
                nc.tensor.matmul(
                    pd[:, ci : ci + 1],
                    lhsT=s_prod[:, ci * 128 : (ci + 1) * 128],
                    rhs=s_ones[:],
                    start=True,
                    stop=True,
                )
            s_d = const.tile([128, NCHUNK], FP32)
            nc.vector.tensor_copy(s_d[:], pd[:])

            # ---- stage 6: S blocks (bf16) + softplus row-sums
            # softplus(S) = relu(S) + log1p(exp(-|S|));
            # row-sum accumulated per chunk into s_pos.
            for grp in range(NH):
                # chunk stride padded to 256 so no matmul crosses a PSUM bank
                pS = psum_sp.tile([128, CPH * 2, 256], FP32, tag="spsum")
                for gg in range(CPH):
                    g = grp * CPH + gg
                    for h in range(2):
                        nc.tensor.matmul(
                            pS[:, 2 * gg + h, :JW],
                            lhsT=s_u16[:, g * B + h * 128 : g * B + (h + 1) * 128],
                            rhs=s_fz16[:, g * B : g * B + JW],
                            start=True,
                            stop=True,
                        )
                nch = CPH * 2
                s_a = junkp.tile([128, nch * JW], FP32, tag="abs")
                nc.scalar.activation(
                    out=s_a.rearrange("p (c j) -> p c j", j=JW),
                    in_=pS[:, :, :JW],
                    func=F.Abs,
                )
                s_e = junkp.tile([128, nch * JW], FP32, tag="exp")
                nc.scalar.activation(out=s_e[:], in_=s_a[:], func=F.Exp, scale=-1.0)
                s_l = junkp.tile([128, nch * JW], FP32, tag="l1p")
                nc.scalar.activation(out=s_l[:], in_=s_e[:], func=F.Ln, bias=1.0)
                for ch in range(nch):
                    ci = grp * nch + ch
                    jk = junkp.tile([128, JW], FP32, tag="junk")
                    nc.vector.scalar_tensor_tensor(
                        out=jk[:],
                        in0=pS[:, ch, :JW],
                        scalar=0.0,
                        in1=s_l[:, ch * JW : (ch + 1) * JW],
                        op0=OP.max,
                        op1=OP.add,
                        accum_out=s_pos[:, ci : ci + 1],
                    )

            # ---- stage 7: accurate T-term: softplus(d) via exp/log1p
            sm = const
            s_ad = sm.tile([128, NCHUNK], FP32)
            nc.scalar.activation(out=s_ad[:], in_=s_d[:], func=F.Abs)
            s_ed = sm.tile([128, NCHUNK], FP32)
            nc.scalar.activation(out=s_ed[:], in_=s_ad[:], func=F.Exp, scale=-1.0)
            s_big = sm.tile([128, NCHUNK], FP32)
            nc.scalar.activation(out=s_big[:], in_=s_ed[:], func=F.Ln, bias=1.0)
            s_t1 = sm.tile([128, NCHUNK], FP32)
            nc.vector.tensor_scalar(
                out=s_t1[:], in0=s_ed[:], scalar1=-0.5, scalar2=1.0,
                op0=OP.mult, op1=OP.add,
            )  # 1 - e/2
            s_ssm = sm.tile([128, NCHUNK], FP32)
            nc.vector.tensor_mul(s_ssm[:], s_ed[:], s_t1[:])  # e*(1 - e/2)
            s_mk = sm.tile([128, NCHUNK], mb.dt.int8)
            nc.vector.tensor_scalar(
                out=s_mk[:], in0=s_ed[:], scalar1=1e-4, scalar2=None, op0=OP.is_lt
            )
            s_l1p = sm.tile([128, NCHUNK], FP32)
            nc.vector.select(s_l1p[:], s_mk[:], s_ssm[:], s_big[:])
            s_T0 = sm.tile([128, NCHUNK], FP32)
            nc.vector.scalar_tensor_tensor(
                out=s_T0[:], in0=s_d[:], scalar=0.0, in1=s_l1p[:],
                op0=OP.max, op1=OP.add,
            )  # relu(d) + log1p(exp(-|d|))
            s_logT = sm.tile([128, NCHUNK], FP32)
            nc.scalar.activation(out=s_logT[:], in_=s_T0[:], func=F.Ln, bias=s_eps[:])

            # ---- stage 8: neg_T and final output
            s_q1 = sm.tile([128, NCHUNK], FP32)
            nc.vector.tensor_sub(s_q1[:], s_pos[:], s_subv[:])
            s_q2 = sm.tile([128, NCHUNK], FP32)
            nc.vector.tensor_mul(s_q2[:], s_q1[:], s_pinv[:])
            s_q2c = sm.tile([128, NCHUNK], FP32)
            nc.vector.tensor_scalar_max(s_q2c[:], s_q2[:], 0.0)
            s_logn = sm.tile([128, NCHUNK], FP32)
            nc.scalar.activation(
                out=s_logn[:], in_=s_q2c[:], func=F.Ln, bias=s_eps[:]
            )
            s_y = sm.tile([128, NCHUNK], FP32)
            nc.vector.tensor_sub(s_y[:], s_logT[:], s_logn[:])

            nc.sync.dma_start(
                out=d_yout.rearrange("(c p) -> p c", p=128), in_=s_y[:]
            )

    nc.compile()
    return nc


def get_program():
    global _PROGRAM
    if _PROGRAM is None:
        _PROGRAM = _build_program()
    return _PROGRAM


# ---------------------------------------------------------------- host side
def _prep_core_inputs(x, z, Ws, idx_lists, core):
    """Build the per-core input map (grouped, padded, transposed)."""
    xgT = np.zeros((IN, R), np.float32)
    zgT = np.zeros((Z, R), np.float32)
    mrow = np.zeros((1, R), np.float32)
    subv = np.zeros(R, np.float32)
    pinv = np.zeros(R, np.float32)
    for s in range(G):
        k = core * G + s
        idx = idx_lists[k]
        n = len(idx)
        lo = s * B
        if n:
            xgT[:, lo : lo + n] = x[idx].T
            zgT[:, lo : lo + n] = z[idx].T
            mrow[0, lo : lo + n] = 1.0
            subv[lo : lo + B] = (JW - n) * LN2
            pinv[lo : lo + B] = 1.0 / n
    wsg = np.ascontiguousarray(
        Ws[core * G : (core + 1) * G].reshape(G * Z, Z), dtype=np.float32
    )
    # chunk layout [128, NCHUNK]: row r = ci*128 + p  ->  [p, ci]
    subv = np.ascontiguousarray(subv.reshape(NCHUNK, 128).T)
    pinv = np.ascontiguousarray(pinv.reshape(NCHUNK, 128).T)
    return {
        "xgT": xgT, "zgT": zgT, "mrow": mrow, "wsg": wsg,
        "subv": subv, "pinv": pinv,
    }


def _numpy_fallback(x, c, z, W1, b1, W2, b2, Wz, bz, Ws):
    x64 = x.astype(np.float64)
    fx = np.maximum(x64 @ W1.astype(np.float64) + b1, 0.0) @ W2.astype(
        np.float64
    ) + b2
    fz = z.astype(np.float64) @ Wz.astype(np.float64) + bz
    u = np.einsum("nd,nde->ne", fx, Ws.astype(np.float64)[c])

    def sp(v):
        return np.log1p(np.exp(-np.abs(v))) + np.maximum(v, 0.0)

    T = sp(np.einsum("ne,ne->n", u, fz))
    out = np.empty(N, np.float64)
    for k in range(C):
        idx = np.where(c == k)[0]
        if len(idx) == 0:
            continue
        Sk = sp(u[idx] @ fz[idx].T)
        neg = Sk.mean(axis=1)
        out[idx] = np.log(T[idx] + EPS) - np.log(neg + EPS)
    return out.astype(np.float32)


def kernel(x, c, z, W1, b1, W2, b2, Wz, bz, Ws):
    x = np.ascontiguousarray(np.asarray(x), dtype=np.float32)
    z = np.ascontiguousarray(np.asarray(z), dtype=np.float32)
    W1 = np.ascontiguousarray(np.asarray(W1), dtype=np.float32)
    b1 = np.ascontiguousarray(np.asarray(b1), dtype=np.float32)
    W2 = np.ascontiguousarray(np.asarray(W2), dtype=np.float32)
    b2 = np.ascontiguousarray(np.asarray(b2), dtype=np.float32)
    Wz = np.ascontiguousarray(np.asarray(Wz), dtype=np.float32)
    bz = np.ascontiguousarray(np.asarray(bz), dtype=np.float32)
    Ws = np.ascontiguousarray(np.asarray(Ws), dtype=np.float32)
    cf = np.asarray(c).reshape(-1).astype(np.int64)

    idx_lists = [np.where(cf == k)[0] for k in range(C)]
    if max(len(i) for i in idx_lists) > JW:
        return _numpy_fallback(x, cf, z, W1, b1, W2, b2, Wz, bz, Ws)

    shared = {
        "w1": W1,
        "b1": b1.reshape(H, 1),
        "w2": W2,
        "b2": b2.reshape(Z, 1),
        "wz": Wz,
        "bz": bz.reshape(1, Z),
    }
    in_maps = []
    for core in range(NCORES):
        m = _prep_core_inputs(x, z, Ws, idx_lists, core)
        m.update(shared)
        in_maps.append(m)

    nc = get_program()
    res = run_bass_kernel_spmd(nc, in_maps, core_ids=list(range(NCORES)))

    out = np.empty(N, np.float32)
    for core in range(NCORES):
        y = res.results[core]["yout"]
        for s in range(G):
            k = core * G + s
            idx = idx_lists[k]
            if len(idx):
                out[idx] = y[s * B : s * B + len(idx)]
    return out
